# revision 13
# baseline (speedup 1.0000x reference)
"""Trainium2 Bass kernel: e3nn edge message block (gnn_message_passing).

Strategy (edge-parallel across 8 cores):
  - Host: fold norm constants into weights, block-diagonal MLP weights so one
    matmul serves two 512-edge tiles, feature-major layouts, shard edges
    25000/core (padded to 50*512).
  - Device phase A: linear_up as 80 wide feature-major matmuls; XBAR DMA
    transpose into the node-major SBUF table Tn[u, n, :] = [s|vx|vy|vz].
  - Device phase B, per PAIR of 512-edge tiles:
      * gpsimd.dma_gather pulls per-edge sender rows from the SBUF table
      * y0/y1 per-edge scalars broadcast across partitions by DMA (stride-0)
      * radial MLP once per pair (block-diag weights, both tiles stacked in
        the partition dim), silu on ACT
      * uvu tensor product: psum evacuated wide on ACT, elementwise muls as
        flat 2D bf16 DVE ops (fast mode)
      * final linear: 8 matmuls into one [128, 4*512] PSUM tile, single ACT
        evacuation, bf16 output written feature-major; host transposes back
"""

import os
import sys

sys.path.insert(0, "/opt/trn_rl_repo")

import numpy as np

MUL = 128
N_NODES = 10000
N_NODES_PAD = 10240              # 10 chunks of 1024 for phase A
N_EDGES = 200000
N_CORES = 8
ES = N_EDGES // N_CORES          # 25000 edges per core
F = 512                          # edges per tile (free dim)
NT = 50                          # tiles per core (even, for pairing)
ESP = NT * F                     # 25600 padded edges per core
EDGE_FEAT_DIM = 8
HIDDEN = 64


def _silu_cst():
    z = np.linspace(-12.0, 12.0, 200001)
    pdf = np.exp(-0.5 * z * z) / np.sqrt(2.0 * np.pi)
    silu = z / (1.0 + np.exp(-z))
    trapz = getattr(np, "trapezoid", None) or getattr(np, "trapz")
    return np.float32(1.0 / np.sqrt(trapz(silu * silu * pdf, z)))


def build_program(f=F, nt=NT, pair_chunks=None):
    """Build the SPMD single-core Bass program (same program on all cores).

    pair_chunks[p] = number of 1024-node table chunks pair p's gathers need
    (edges are host-sorted by sender, so pair p only reads a prefix of Tn).
    """
    import concourse.bass as bass
    import concourse.bacc as bacc
    import concourse.tile as tile
    from concourse import mybir

    f32 = mybir.dt.float32
    bf16 = mybir.dt.bfloat16
    i16 = mybir.dt.int16
    AF = mybir.ActivationFunctionType

    esp = nt * f
    npairs = nt // 2
    nchunk = N_NODES_PAD // 1024     # 10
    nblk = N_NODES_PAD // 128        # 80
    nc = bacc.Bacc(None, target_bir_lowering=False, debug=False)

    # ---- DRAM parameters --------------------------------------------------
    nfT = nc.declare_dram_parameter("nfT", [4 * MUL, N_NODES_PAD], bf16, isOutput=False)
    idx_d = nc.declare_dram_parameter("idx", [128, nt * (f // 16)], i16, isOutput=False)
    ef2_d = nc.declare_dram_parameter("ef2", [2 * EDGE_FEAT_DIM, npairs * f], bf16, isOutput=False)
    yT_d = nc.declare_dram_parameter("yT", [1, 4 * esp], bf16, isOutput=False)
    W1bd_d = nc.declare_dram_parameter("W1bd", [2 * EDGE_FEAT_DIM, 2 * HIDDEN], bf16, isOutput=False)
    W2bd_d = nc.declare_dram_parameter("W2bd", [2 * HIDDEN, 2 * HIDDEN], bf16, isOutput=False)
    W3bd_d = nc.declare_dram_parameter("W3bd", [2 * HIDDEN, 2 * HIDDEN], bf16, isOutput=False)
    W42_d = nc.declare_dram_parameter("W42", [128, 4 * MUL], bf16, isOutput=False)
    WupS_d = nc.declare_dram_parameter("WupS", [MUL, MUL], bf16, isOutput=False)
    WupV_d = nc.declare_dram_parameter("WupV", [MUL, MUL], bf16, isOutput=False)
    Wout_d = nc.declare_dram_parameter("Wout", [MUL, 4 * MUL], bf16, isOutput=False)
    outT_d = nc.declare_dram_parameter("outT", [4 * MUL, esp], bf16, isOutput=True)

    with tile.TileContext(nc) as tc:
        with (
            tc.tile_pool(name="const", bufs=1) as const,
            tc.tile_pool(name="tables", bufs=1) as tabs,
            tc.tile_pool(name="work", bufs=2) as work,
            tc.tile_pool(name="psum", bufs=2, space="PSUM") as psum,
        ):
            # ---- constants into SBUF -------------------------------------
            def cload(dram, shape, name):
                t = const.tile(shape, bf16, name=name, tag=name)
                nc.sync.dma_start(out=t[:], in_=dram[:])
                return t

            W1bd_s = cload(W1bd_d, [2 * EDGE_FEAT_DIM, 2 * HIDDEN], "cW1")
            W2bd_s = cload(W2bd_d, [2 * HIDDEN, 2 * HIDDEN], "cW2")
            W3bd_s = cload(W3bd_d, [2 * HIDDEN, 2 * HIDDEN], "cW3")
            W42_s = cload(W42_d, [128, 4 * MUL], "cW4")
            WupS_s = cload(WupS_d, [MUL, MUL], "cWupS")
            WupV_s = cload(WupV_d, [MUL, MUL], "cWupV")
            Wout_s = cload(Wout_d, [MUL, 4 * MUL], "cWout")  # A|B|C|D blocks
            idx_s = const.tile([128, nt * (f // 16)], i16, name="cidx", tag="cidx")
            nc.sync.dma_start(out=idx_s[:], in_=idx_d[:])

            A_s = Wout_s[:, 0:MUL]
            B_s = Wout_s[:, MUL : 2 * MUL]
            C_s = Wout_s[:, 2 * MUL : 3 * MUL]
            D_s = Wout_s[:, 3 * MUL : 4 * MUL]

            # ---- phase A: node-major table via wide matmuls + XBAR -------
            # Tn[p, blk, :] = [s | vx | vy | vz] row of node (blk*128 + p)
            Tn = tabs.tile([128, nblk, 4 * MUL], bf16)
            wcfg = [WupS_s, WupV_s, WupV_s, WupV_s]
            if pair_chunks is None:
                pair_chunks = [nchunk] * npairs

            def emit_chunk(c):
                n0 = c * 1024
                xa = work.tile([128, 4, 1024], bf16, tag="nfc", bufs=2)
                nc.sync.dma_start(
                    out=xa[:],
                    in_=nfT.rearrange("(k p) n -> p k n", k=4)[:, :, n0 : n0 + 1024],
                )
                for k in range(4):
                    pk = psum.tile([128, 1024], f32, tag="psO", bufs=2, name="pk")
                    for h in range(2):
                        nc.tensor.matmul(
                            pk[:, 512 * h : 512 * (h + 1)],
                            lhsT=wcfg[k][:],
                            rhs=xa[:, k, 512 * h : 512 * (h + 1)],
                            start=True,
                            stop=True,
                        )
                    pf = work.tile([128, 1024], bf16, tag="pf", bufs=3, name="pf")
                    nc.scalar.activation(pf[:], pk[:], AF.Copy)
                    eng = nc.scalar if k == 3 else nc.sync
                    eng.dma_start(
                        out=Tn[:, c * 8 : (c + 1) * 8, 128 * k : 128 * (k + 1)],
                        in_=pf[:],
                        transpose=True,
                    )

            chunks_done = 0

            # ---- phase B: edge tile pairs, A/B halves interleaved --------
            for p in range(npairs):
                while chunks_done < pair_chunks[p]:
                    emit_chunk(chunks_done)
                    chunks_done += 1
                tA, tB = 2 * p, 2 * p + 1

                def gather(t):
                    G = work.tile([128, 4, f], bf16, tag="G1", bufs=3)
                    nc.gpsimd.dma_gather(
                        G[:],
                        Tn[:, 0 : 8 * pair_chunks[p], :],
                        idx_s[:, t * (f // 16) : (t + 1) * (f // 16)],
                        num_idxs=f,
                        num_idxs_reg=f,
                        elem_size=4 * MUL,
                        transpose=True,
                        sbuf_tokens_per_rank=128,
                        sbuf_free_dim_per_rank=4 * MUL * 2,
                        sbuf_free_dim_pad_per_rank=0,
                        sbuf_byte_offset=0,
                    )
                    return G

                def ybcast(t):
                    y = work.tile([128, 4, f], bf16, tag="y4", bufs=3)
                    nc.sync.dma_start(
                        out=y[:],
                        in_=yT_d[0:1, 4 * t * f : 4 * (t + 1) * f].partition_broadcast(128),
                    )
                    return y

                G1A, G1B = gather(tA), gather(tB)
                y4A, y4B = ybcast(tA), ybcast(tB)

                etp = work.tile([2 * EDGE_FEAT_DIM, f], bf16, tag="et", bufs=3)
                nc.sync.dma_start(out=etp[:], in_=ef2_d[:, p * f : (p + 1) * f])

                # radial MLP for both tiles at once (block-diag weights)
                ph1 = psum.tile([2 * HIDDEN, f], f32, tag="ps1", bufs=4)
                nc.tensor.matmul(ph1[:], lhsT=W1bd_s[:], rhs=etp[:], start=True, stop=True)
                h1 = work.tile([2 * HIDDEN, f], bf16, tag="h1", bufs=3)
                nc.scalar.activation(h1[:], ph1[:], AF.Silu)
                ph2 = psum.tile([2 * HIDDEN, f], f32, tag="ps1", bufs=4)
                nc.tensor.matmul(ph2[:], lhsT=W2bd_s[:], rhs=h1[:], start=True, stop=True)
                h2 = work.tile([2 * HIDDEN, f], bf16, tag="h2", bufs=3)
                nc.scalar.activation(h2[:], ph2[:], AF.Silu)
                ph3 = psum.tile([2 * HIDDEN, f], f32, tag="ps1", bufs=4)
                nc.tensor.matmul(ph3[:], lhsT=W3bd_s[:], rhs=h2[:], start=True, stop=True)
                h3 = work.tile([2 * HIDDEN, f], bf16, tag="h3", bufs=3)
                nc.scalar.activation(h3[:], ph3[:], AF.Silu)

                # h3 * y0 per half (y broadcasts cover all 128 partitions)
                h3y0 = work.tile([2 * HIDDEN, f], bf16, tag="h3y0", bufs=3)
                nc.vector.tensor_mul(
                    out=h3y0[0:HIDDEN, :], in0=h3[0:HIDDEN, :], in1=y4A[0:HIDDEN, 0, :]
                )
                nc.vector.tensor_mul(
                    out=h3y0[HIDDEN:, :], in0=h3[HIDDEN:, :], in1=y4B[HIDDEN:, 0, :]
                )

                halves = []
                for (G1, y4, hb, t) in ((G1A, y4A, 0, tA), (G1B, y4B, HIDDEN, tB)):
                    halves.append(dict(G1=G1, y4=y4, hs=slice(hb, hb + HIDDEN), t=t))

                # tpw quadrant matmuls, A/B interleaved (W42 has W4 twice)
                for H in halves:
                    hs = H["hs"]
                    H["pwa"] = psum.tile([128, f], f32, tag="ps1", bufs=4, name="pwa")
                    nc.tensor.matmul(H["pwa"][:], lhsT=W42_s[hs, 0:128], rhs=h3y0[hs, :], start=True, stop=True)
                    H["pwc"] = psum.tile([128, f], f32, tag="ps1", bufs=4, name="pwc")
                    nc.tensor.matmul(H["pwc"][:], lhsT=W42_s[hs, 256:384], rhs=h3[hs, :], start=True, stop=True)
                for H in halves:
                    hs = H["hs"]
                    H["pwb"] = psum.tile([128, f], f32, tag="ps1", bufs=4, name="pwb")
                    nc.tensor.matmul(H["pwb"][:], lhsT=W42_s[hs, 128:256], rhs=h3[hs, :], start=True, stop=True)
                    H["pwd"] = psum.tile([128, f], f32, tag="ps1", bufs=4, name="pwd")
                    nc.tensor.matmul(H["pwd"][:], lhsT=W42_s[hs, 384:512], rhs=h3y0[hs, :], start=True, stop=True)

                # DVE: pprime/zt straight from psum; ACT: evac pwb/pwd
                for H in halves:
                    s1 = H["G1"][:, 0, :]
                    H["pprime"] = work.tile([128, f], bf16, tag="pp", bufs=3, name="pp")
                    nc.vector.tensor_mul(out=H["pprime"][:], in0=H["pwa"][:], in1=s1)
                    H["zt"] = work.tile([128, f], bf16, tag="zt", bufs=3, name="zt")
                    nc.vector.tensor_mul(out=H["zt"][:], in0=H["pwc"][:], in1=s1)
                for H in halves:
                    H["wb"] = work.tile([128, f], bf16, tag="wb", bufs=3, name="wb")
                    nc.scalar.activation(H["wb"][:], H["pwb"][:], AF.Copy)
                    H["wd"] = work.tile([128, f], bf16, tag="wd", bufs=3, name="wd")
                    nc.scalar.activation(H["wd"][:], H["pwd"][:], AF.Copy)

                # dot product chain (flat 2D ops)
                for H in halves:
                    Gf = H["G1"][:].rearrange("p a b -> p (a b)")
                    yf = H["y4"][:].rearrange("p a b -> p (a b)")
                    H["dm"] = work.tile([128, 3 * f], bf16, tag="dm", bufs=3, name="dm")
                    nc.vector.tensor_mul(out=H["dm"][:], in0=Gf[:, f : 4 * f], in1=yf[:, f : 4 * f])
                for H in halves:
                    dm = H["dm"]
                    H["ds"] = work.tile([128, f], bf16, tag="ds", bufs=3, name="ds")
                    nc.vector.tensor_add(out=H["ds"][:], in0=dm[:, 0:f], in1=dm[:, f : 2 * f])
                for H in halves:
                    H["dot"] = work.tile([128, f], bf16, tag="dot", bufs=3, name="dot")
                    nc.vector.tensor_add(out=H["dot"][:], in0=H["ds"][:], in1=H["dm"][:, 2 * f : 3 * f])
                for H in halves:
                    H["rbar"] = work.tile([128, f], bf16, tag="rbar", bufs=3, name="rbar")
                    nc.vector.tensor_mul(out=H["rbar"][:], in0=H["wb"][:], in1=H["dot"][:])

                # q_m = zt*y1m, t_m = wd*v1m (broadcast along plane axis)
                for H in halves:
                    H["q3"] = work.tile([128, 3, f], bf16, tag="q3", bufs=2, name="q3")
                    nc.vector.tensor_mul(
                        out=H["q3"][:], in0=H["y4"][:, 1:4, :],
                        in1=H["zt"][:].unsqueeze(1).broadcast_to([128, 3, f]),
                    )
                    H["t3"] = work.tile([128, 3, f], bf16, tag="t3", bufs=2, name="t3")
                    nc.vector.tensor_mul(
                        out=H["t3"][:], in0=H["G1"][:, 1:4, :],
                        in1=H["wd"][:].unsqueeze(1).broadcast_to([128, 3, f]),
                    )

                # final linear: two 2-plane psum groups per tile, interleaved
                for H in halves:
                    H["outO"] = work.tile([128, 4, f], bf16, tag="oO", bufs=2, name="oO")
                for grp in range(2):
                    for H in halves:
                        ps = psum.tile([128, 2, f], f32, tag="psO", bufs=2, name="psg")
                        if grp == 0:
                            nc.tensor.matmul(ps[:, 0, :], lhsT=A_s, rhs=H["pprime"][:], start=True, stop=False)
                            nc.tensor.matmul(ps[:, 0, :], lhsT=B_s, rhs=H["rbar"][:], start=False, stop=True)
                            nc.tensor.matmul(ps[:, 1, :], lhsT=C_s, rhs=H["q3"][:, 0, :], start=True, stop=False)
                            nc.tensor.matmul(ps[:, 1, :], lhsT=D_s, rhs=H["t3"][:, 0, :], start=False, stop=True)
                        else:
                            for m in (1, 2):
                                nc.tensor.matmul(ps[:, m - 1, :], lhsT=C_s, rhs=H["q3"][:, m, :], start=True, stop=False)
                                nc.tensor.matmul(ps[:, m - 1, :], lhsT=D_s, rhs=H["t3"][:, m, :], start=False, stop=True)
                        dst = H["outO"][:, 2 * grp : 2 * grp + 2, :].rearrange("p a b -> p (a b)")
                        nc.scalar.activation(dst, ps[:].rearrange("p a b -> p (a b)"), AF.Copy)
                for H in halves:
                    e0 = H["t"] * f
                    nc.sync.dma_start(
                        out=outT_d.rearrange("(r p) e -> p r e", r=4)[:, :, e0 : e0 + f],
                        in_=H["outO"][:],
                    )

            while chunks_done < nchunk:
                emit_chunk(chunks_done)
                chunks_done += 1

    nc.compile()
    return nc


def prep_host_inputs(node_feats, edge_index, edge_attrs, edge_feats,
                     W_up_s, W_up_v, W1, W2, W3, W4, W_out_s, W_out_v,
                     f=F, nt=NT, n_cores=N_CORES):
    """Fold constants, build device layouts, shard edges. Returns in_maps."""
    import ml_dtypes

    cst = _silu_cst()
    node_feats = np.asarray(node_feats, dtype=np.float32)
    edge_attrs = np.asarray(edge_attrs, dtype=np.float32)
    edge_feats = np.asarray(edge_feats, dtype=np.float32)
    sender = np.asarray(edge_index)[0].astype(np.int64)

    esp = nt * f
    npairs = nt // 2
    n_edges = sender.shape[0]
    es = n_edges // n_cores
    assert es % (2 * f) == 0 or True

    # weights with all norm constants folded
    W1h = (np.asarray(W1, np.float32) / np.sqrt(np.float32(EDGE_FEAT_DIM)))
    W2h = (np.asarray(W2, np.float32) / np.sqrt(np.float32(HIDDEN))) * cst
    W3h = (np.asarray(W3, np.float32) / np.sqrt(np.float32(HIDDEN))) * cst
    W4h = (np.asarray(W4, np.float32) / np.sqrt(np.float32(HIDDEN))) * cst

    def blockdiag(W):
        n, m = W.shape
        out = np.zeros((2 * n, 2 * m), np.float32)
        out[:n, :m] = W
        out[n:, m:] = W
        return out

    W1bd = blockdiag(W1h)
    W2bd = blockdiag(W2h)
    W3bd = blockdiag(W3h)
    W42 = np.concatenate([W4h, W4h], axis=0)        # [128, 512], both halves

    inv_sqrt_mul = np.float32(1.0 / np.sqrt(MUL))
    WupSh = np.asarray(W_up_s, np.float32) * inv_sqrt_mul
    WupVh = np.asarray(W_up_v, np.float32) * inv_sqrt_mul
    inv2 = np.float32(1.0 / np.sqrt(2 * MUL))
    A = np.asarray(W_out_s, np.float32)[:MUL] * inv2
    B = np.asarray(W_out_s, np.float32)[MUL:] * (inv2 / np.sqrt(np.float32(3.0)))
    C = np.asarray(W_out_v, np.float32)[:MUL] * inv2
    D = np.asarray(W_out_v, np.float32)[MUL:] * inv2
    Wout = np.concatenate([A, B, C, D], axis=1)

    # node features, feature-major planes: s, vx, vy, vz (padded nodes)
    nfT = np.zeros((4, MUL, N_NODES_PAD), np.float32)
    nfT[0, :, :N_NODES] = node_feats[:, :MUL].T
    for m in range(3):
        nfT[1 + m, :, :N_NODES] = node_feats[:, MUL + m :: 3].T
    nfT = np.ascontiguousarray(nfT.reshape(4 * MUL, N_NODES_PAD))

    bf = ml_dtypes.bfloat16
    shared = {
        "nfT": np.ascontiguousarray(nfT.astype(bf)),
        "W1bd": np.ascontiguousarray(W1bd.astype(bf)),
        "W2bd": np.ascontiguousarray(W2bd.astype(bf)),
        "W3bd": np.ascontiguousarray(W3bd.astype(bf)),
        "W42": np.ascontiguousarray(W42.astype(bf)),
        "WupS": np.ascontiguousarray(WupSh.astype(bf)),
        "WupV": np.ascontiguousarray(WupVh.astype(bf)),
        "Wout": np.ascontiguousarray(Wout.astype(bf)),
    }

    in_maps = []
    orders = []
    pair_bound = np.zeros(npairs, np.int64)
    for c in range(n_cores):
        lo, hi = c * es, (c + 1) * es
        order = np.argsort(sender[lo:hi], kind="stable")
        orders.append(order)
        snd = np.zeros(esp, np.int16)
        snd[: es] = sender[lo:hi][order].astype(np.int16)
        pb = snd.reshape(npairs, 2 * f).max(axis=1)
        pair_bound = np.maximum(pair_bound, pb)
        # ap_gather layout: idx[16g+p, t*(f//16)+s] = snd[t*f + s*16 + p]
        sp = snd.reshape(nt, f // 16, 16)           # [t, s, p]
        grid16 = sp.transpose(2, 0, 1).reshape(16, nt * (f // 16))
        idx_l = np.ascontiguousarray(np.tile(grid16, (8, 1)))

        efT = np.zeros((EDGE_FEAT_DIM, esp), np.float32)
        efT[:, :es] = edge_feats[lo:hi][order].T
        # pair-stacked layout: ef2[0:8, p*f+e] = tile 2p, ef2[8:16,...] = tile 2p+1
        ef2 = np.ascontiguousarray(
            efT.reshape(EDGE_FEAT_DIM, npairs, 2, f).transpose(2, 0, 1, 3)
            .reshape(2 * EDGE_FEAT_DIM, npairs * f)
        ).astype(bf)

        yT = np.zeros((4, esp), np.float32)
        yT[:, :es] = edge_attrs[lo:hi][order].T
        # per-tile flat layout: [1, t*4f + r*f + e]
        y_flat = np.ascontiguousarray(
            yT.reshape(4, nt, f).transpose(1, 0, 2).reshape(1, 4 * esp)
        ).astype(bf)

        in_maps.append(dict(shared, idx=idx_l, ef2=ef2, yT=y_flat))
    pair_chunks = np.maximum.accumulate(
        np.ceil((pair_bound + 1) / 1024.0).astype(np.int64)
    )
    pair_chunks = np.clip(pair_chunks, 1, N_NODES_PAD // 1024)
    return in_maps, tuple(int(x) for x in pair_chunks), orders


_PROG_CACHE = {}


def _parse_hw_time_ns(profile_dir):
    """Max over cores of neuron-profile total_time for the kernel NEFF."""
    import glob
    import json
    import shutil
    import subprocess

    if not shutil.which("neuron-profile"):
        return None
    neffs = glob.glob(os.path.join(profile_dir, "*_body*.neff"))
    ntffs = sorted(glob.glob(os.path.join(profile_dir, "*_body*-device*.ntff")))
    if not neffs or not ntffs:
        return None
    times = []
    for ntff in ntffs:
        try:
            out = subprocess.run(
                ["neuron-profile", "view", "-n", neffs[0], "-s", ntff,
                 "--output-format", "summary-json"],
                capture_output=True, text=True, timeout=120,
            ).stdout
            payload = json.loads(out)
            v = next(iter(payload.values()))
            times.append(float(v["total_time"]))
        except Exception as e:  # noqa: BLE001
            print(f"profile parse failed for {os.path.basename(ntff)}: {e}")
    if not times:
        return None
    print("per-core HW us:", [f"{t*1e6:.1f}" for t in times])
    return int(max(times) * 1e9)


def _run_pjrt(nc, in_maps, n_cores=N_CORES, time_reps=0, profile_dir=None):
    """Execute the SPMD program via PJRT. Returns (results, wall_times, hw_ns)."""
    import time as _time

    import jax
    from jax.sharding import Mesh, NamedSharding, PartitionSpec

    try:
        from jax.experimental.shard_map import shard_map
    except ImportError:  # newer jax
        from jax.sharding import shard_map
    from concourse import bass2jax, mybir

    bass2jax.install_neuronx_cc_hook()

    save_neff = os.environ.get("KERNEL_SAVE_NEFF")
    if save_neff:
        _orig_rename = bass2jax.rename_neff_tensors_and_patch_header.__wrapped__ if hasattr(
            bass2jax.rename_neff_tensors_and_patch_header, "__wrapped__"
        ) else bass2jax.rename_neff_tensors_and_patch_header

        def _rename_and_save(neff_file, renames):
            data = _orig_rename(neff_file, renames)
            with open(save_neff, "wb") as fh:
                fh.write(data)
            return data

        bass2jax.rename_neff_tensors_and_patch_header = _rename_and_save

    partition_name = (
        nc.partition_id_tensor.name if nc.partition_id_tensor is not None else None
    )
    in_names, out_names, out_avals, zero_outs = [], [], [], []
    for alloc in nc.m.functions[0].allocations:
        if not isinstance(alloc, mybir.MemoryLocationSet):
            continue
        name = alloc.memorylocations[0].name
        if alloc.kind == "ExternalInput":
            if name != partition_name:
                in_names.append(name)
        elif alloc.kind == "ExternalOutput":
            shape = tuple(alloc.tensor_shape)
            dtype = mybir.dt.np(alloc.dtype)
            out_names.append(name)
            out_avals.append(jax.core.ShapedArray(shape, dtype))
            zero_outs.append(np.zeros(shape, dtype))
    n_params = len(in_names)
    in_names_all = in_names + out_names
    if partition_name is not None:
        in_names_all = in_names_all + [partition_name]

    def _body(*args):
        operands = list(args)
        if partition_name is not None:
            operands.append(bass2jax.partition_id_tensor())
        outs = bass2jax._bass_exec_p.bind(
            *operands,
            out_avals=tuple(out_avals),
            in_names=tuple(in_names_all),
            out_names=tuple(out_names),
            lowering_input_output_aliases=(),
            sim_require_finite=True,
            sim_require_nnan=True,
            nc=nc,
        )
        return tuple(outs)

    devices = jax.devices()[:n_cores]
    mesh = Mesh(np.asarray(devices), ("core",))
    nouts = len(out_names)
    donate = tuple(range(n_params, n_params + nouts))

    spec = NamedSharding(mesh, PartitionSpec("core"))
    dev_in = [
        jax.device_put(
            np.concatenate([np.asarray(in_maps[c][nm]) for c in range(n_cores)], axis=0),
            spec,
        )
        for nm in in_names
    ]

    def make_zeros():
        return [
            jax.device_put(np.zeros((n_cores * z.shape[0], *z.shape[1:]), z.dtype), spec)
            for z in zero_outs
        ]

    def _compile():
        return (
            jax.jit(
                shard_map(
                    _body,
                    mesh=mesh,
                    in_specs=(PartitionSpec("core"),) * (n_params + nouts),
                    out_specs=(PartitionSpec("core"),) * nouts,
                    check_rep=False,
                ),
                donate_argnums=donate,
                keep_unused=True,
            )
            .lower(*dev_in, *make_zeros())
            .compile()
        )

    try:
        sharded = bass2jax.fast_dispatch_compile(_compile)
    except Exception as e:  # noqa: BLE001
        print(f"fast_dispatch_compile unavailable ({e}); plain jit")
        sharded = _compile()

    out_arrs = jax.block_until_ready(sharded(*dev_in, *make_zeros()))

    # Timed reps: the kernel writes every output element, so the previous
    # rep's outputs serve as the next rep's donated output buffers — no
    # host->device traffic between reps.
    times = []
    for _ in range(max(time_reps, 0)):
        t0 = _time.perf_counter()
        out_arrs = jax.block_until_ready(sharded(*dev_in, *out_arrs))
        times.append(_time.perf_counter() - t0)

    hw_ns = None
    if profile_dir:
        try:
            prof = _ntff_profiler()
        except Exception as e:  # noqa: BLE001
            print(f"profiler unavailable: {e}")
            prof = None
        if prof is not None:
            # profiled extra rep (excluded from wall times)
            final = sharded(*dev_in, *out_arrs)
            jax.block_until_ready(final)
            prof.start()
            out_arrs = jax.block_until_ready(sharded(*dev_in, *final))
            prof.stop(profile_dir)
            hw_ns = _parse_hw_time_ns(profile_dir)

    results = [
        {
            nm: np.asarray(out_arrs[i]).reshape(n_cores, *out_avals[i].shape)[c]
            for i, nm in enumerate(out_names)
        }
        for c in range(n_cores)
    ]
    return results, times, hw_ns


class _ntff_profiler:
    def __init__(self, so_path="/opt/axon/libaxon_pjrt.so"):
        import ctypes

        self.lib = ctypes.CDLL(so_path)
        self.ctypes = ctypes
        self.lib.axon_start_nrt_profile.argtypes = [
            ctypes.POINTER(ctypes.c_int64),
            ctypes.c_size_t,
        ]
        self.lib.axon_start_nrt_profile.restype = ctypes.c_int64
        self.lib.axon_stop_nrt_profile.argtypes = [ctypes.c_char_p]
        self.lib.axon_stop_nrt_profile.restype = ctypes.c_int64

    def start(self):
        rc = self.lib.axon_start_nrt_profile(None, 0)
        if rc != 0:
            print(f"ntff profile start failed rc={rc}")

    def stop(self, outdir):
        os.makedirs(outdir, exist_ok=True)
        n = self.lib.axon_stop_nrt_profile(str(outdir).encode())
        print(f"ntff profile: {n} file(s) -> {outdir}")


def kernel(node_feats, edge_index, edge_attrs, edge_feats,
           W_up_s, W_up_v, W1, W2, W3, W4, W_out_s, W_out_v):
    import shutil
    import tempfile

    in_maps, pair_chunks, orders = prep_host_inputs(
        node_feats, edge_index, edge_attrs, edge_feats,
        W_up_s, W_up_v, W1, W2, W3, W4, W_out_s, W_out_v,
    )

    key = (F, NT, pair_chunks)
    if key not in _PROG_CACHE:
        _PROG_CACHE[key] = build_program(F, NT, list(pair_chunks))
    nc = _PROG_CACHE[key]

    time_reps = int(os.environ.get("KERNEL_TIME_REPS", "3"))
    profile_dir = os.environ.get("KERNEL_PROFILE_DIR") or None
    tmp_prof = None
    if profile_dir is None and os.environ.get("KERNEL_NO_PROFILE", "0") != "1":
        tmp_prof = tempfile.mkdtemp(prefix="kprof_")
        profile_dir = tmp_prof
    results, times, hw_ns = _run_pjrt(
        nc, in_maps, N_CORES, time_reps=time_reps, profile_dir=profile_dir
    )
    if tmp_prof is not None:
        shutil.rmtree(tmp_prof, ignore_errors=True)
    if times:
        print(f"wall times (s): {[f'{x:.6f}' for x in times]}")
        kernel.last_wall_times = times
    if hw_ns is not None:
        kernel.last_exec_time_ns = hw_ns
        print(f"profiled HW exec time: {hw_ns} ns")
    elif times:
        kernel.last_exec_time_ns = int(min(times) * 1e9)

    out = np.empty((N_EDGES, 4 * MUL), np.float32)
    for c in range(N_CORES):
        ot = np.asarray(results[c]["outT"], np.float32)[:, :ES]   # [512, ES]
        rows = c * ES + orders[c]
        out[rows, :MUL] = ot[:MUL].T
        out[rows, MUL:] = (
            ot[MUL:].reshape(3, MUL, ES).transpose(2, 1, 0).reshape(ES, 3 * MUL)
        )
    return out
